# revision 77
# baseline (speedup 1.0000x reference)
"""Trainium2 Bass kernel for KG-enhanced embedding model (gnn_message_passing).

Computes, for full inputs:
    inputs_embeds = word_embedding[input_ids]                       # [B,S,H] gather
    h   = relu(entity_embeddings @ W1 + b1)                         # [B,E,MLP_HID]
    ent = h @ W2 + b2                                               # [B,E,H]
    out = inputs_embeds + einsum('bes,beh->bsh', entity_mask, ent)  # masked scatter-add

Sharding: data-parallel over batch B=32 -> 4 examples per core on 8 cores.
Weights and the vocab table are replicated; the gather reads only the rows
each core needs via indirect DMA (16 x 128-row gathers per core).

Matmuls run in float32r (TF32-like: 11 explicit mantissa bits, fp32
accumulate) which streams the PE at 1 cycle/row for N>=256, ~4x faster
than fp32. The mask is 0/1 (exact); weights/activations are pre-rounded to
the f32r grid (round-to-nearest-even at 11 bits) so device matmuls are
deterministic. End-to-end absmax relative error ~1e-4.

Shapes (hardcoded): V=30522, H=768, B=32, S=512, E=8, KG=100, MH=1000.
"""

import os
import numpy as np
from contextlib import ExitStack

V, H = 30522, 768
B, S, E = 32, 512, 8
KG, MH = 100, 1000
NCORES = 8
BPC = B // NCORES              # examples per core = 4
TOK = BPC * S                  # tokens per core = 2048
NCH = TOK // 128               # 128-token chunks per core = 16
KCH = 8                        # K chunks of 128 for the 1000-dim contraction
NE = BPC * E                   # entities per core = 32

_PROGRAM = None


def _maybe_enable_profiling():
    """Optional NTFF profiling (KERNEL_PROFILE=1): shim antenv.axon_hooks."""
    if os.environ.get("KERNEL_PROFILE") != "1":
        return False
    import sys, types
    try:
        from antenv.axon_hooks import get_axon_ntff_profile_hook  # noqa: F401
        return True
    except ImportError:
        pass
    try:
        from trn_agent_boot.trn_boot import _ntff_profile_via_ctypes
        import antenv
        hook = _ntff_profile_via_ctypes("/opt/axon/libaxon_pjrt.so")
        m = types.ModuleType("antenv.axon_hooks")
        m.get_axon_ntff_profile_hook = lambda: hook
        m.set_axon_ntff_profile_hook = lambda h: None
        sys.modules["antenv.axon_hooks"] = m
        antenv.axon_hooks = m
        return True
    except Exception:
        return False


def _build_program():
    import concourse.bacc as bacc
    import concourse.tile as tile
    from concourse import bass, mybir

    f32 = mybir.dt.float32
    f32r = mybir.dt.float32r
    bf16 = mybir.dt.bfloat16
    i32 = mybir.dt.int32
    RELU = mybir.ActivationFunctionType.Relu
    SUB = mybir.AluOpType.subtract

    nc = bacc.Bacc("TRN2", target_bir_lowering=False, debug=False)

    ids_ap = nc.dram_tensor("idsT", [128, NCH], i32, kind="ExternalInput").ap()
    we_ap = nc.dram_tensor("we", [V, H], f32, kind="ExternalInput").ap()
    # w1ee packs bf16 hi/lo of W1 and eeT side by side (one DMA):
    # [w1hi | w1lo | eehi | eelo]
    WEE = 2 * (MH + NE)
    w1ee_ap = nc.dram_tensor("w1ee", [KG, WEE], bf16, kind="ExternalInput").ap()
    b1c_ap = nc.dram_tensor("b1colT", [128, KCH], f32, kind="ExternalInput").ap()
    # w2p packs bf16 hi/lo chunk-major: [hi(KCH*H) | lo(KCH*H)]
    w2_ap = nc.dram_tensor("w2p", [128, 2 * KCH * H], bf16, kind="ExternalInput").ap()
    # b2o [2, H+NE]: row0 = [b2hi | ones], row1 = [b2lo | ones] -> K=2 bias matmul
    b2o_ap = nc.dram_tensor("b2o", [2, H + NE], bf16, kind="ExternalInput").ap()
    maskT_ap = nc.dram_tensor("maskT2", [2 * NE, TOK], bf16, kind="ExternalInput").ap()
    id32_ap = nc.dram_tensor("id32", [NE, NE], bf16, kind="ExternalInput").ap()
    out_ap = nc.dram_tensor("out", [TOK, H], f32, kind="ExternalOutput").ap()

    with tile.TileContext(nc) as tc, ExitStack() as ctx:
        const = ctx.enter_context(tc.tile_pool(name="const", bufs=1))
        psA = ctx.enter_context(tc.tile_pool(name="psA", bufs=2, space="PSUM"))
        psB = ctx.enter_context(tc.tile_pool(name="psB", bufs=1, space="PSUM"))
        psC = ctx.enter_context(tc.tile_pool(name="psC", bufs=2, space="PSUM"))
        gpool = ctx.enter_context(tc.tile_pool(name="gath", bufs=NCH))
        opool = ctx.enter_context(tc.tile_pool(name="outp", bufs=12))

        # ---- loads. SWDGE (gpsimd) traffic starves HWDGE queues, so the
        # SWDGE FIFO carries only what must beat the gathers: ids and w2.
        # Small weights go on sync/scalar HWDGE (they fill SDMA gaps).
        ids_sb = const.tile([128, NCH], i32)
        nc.gpsimd.dma_start(ids_sb[:], ids_ap[:])
        KQ = KCH // 2
        w2_q = []  # [(hi, lo) per half]
        for q in range(2):
            whi = const.tile([128, KQ * H], bf16, tag=f"w2hi{q}")
            nc.gpsimd.dma_start(whi[:], w2_ap[:, q * KQ * H : (q + 1) * KQ * H])
            wlo = const.tile([128, KQ * H], bf16, tag=f"w2lo{q}")
            nc.gpsimd.dma_start(
                wlo[:], w2_ap[:, (2 + q) * KQ * H : (3 + q) * KQ * H]
            )
            w2_q.append((whi, wlo))
        w1ee_sb = const.tile([KG, WEE], bf16)
        nc.sync.dma_start(w1ee_sb[:], w1ee_ap[:])
        w1_hi = w1ee_sb[:, :MH]
        w1_lo = w1ee_sb[:, MH : 2 * MH]
        ee_hi = w1ee_sb[:, 2 * MH : 2 * MH + NE]
        ee_lo = w1ee_sb[:, 2 * MH + NE : WEE]
        b1_col = const.tile([128, KCH], f32)
        nc.sync.dma_start(b1_col[:], b1c_ap[:])
        b2o_sb = const.tile([2, H + NE], bf16)
        nc.sync.dma_start(b2o_sb[:], b2o_ap[:])
        b2_hl = b2o_sb[:, :H]
        ones2 = b2o_sb[:, H : H + NE]
        maskT_sb = const.tile([2 * NE, TOK], bf16)
        nc.scalar.dma_start(maskT_sb[:], maskT_ap[:])
        id32_sb = const.tile([NE, NE], bf16)
        nc.scalar.dma_start(id32_sb[:], id32_ap[:])
        gts = []
        for g in range(NCH):
            gt = gpool.tile([128, H], f32)
            nc.gpsimd.indirect_dma_start(
                out=gt[:],
                out_offset=None,
                in_=we_ap[:],
                in_offset=bass.IndirectOffsetOnAxis(ap=ids_sb[:, g : g + 1], axis=0),
            )
            gts.append(gt)

        # ---- MLP stage 1: hT[k*128+p, e] = relu(W1.T @ ee.T + b1) ----------
        # 3-term bf16 split product; b1 via the activation bias port. The
        # f32 relu output is split into bf16 hi/lo for mm2.
        hT_hi = const.tile([128, KCH, NE], bf16)
        hT_lo = const.tile([128, KCH, NE], bf16)
        hF = const.tile([128, KCH, NE], f32)
        nc.vector.memset(hT_hi[96:128, KCH - 1, :], 0.0)
        nc.vector.memset(hT_lo[96:128, KCH - 1, :], 0.0)
        for k in range(KCH):
            mw = 128 if k < KCH - 1 else MH - 128 * (KCH - 1)  # 104 in last
            ps = psA.tile([128, NE], f32, tag="ps")
            for term, (lt, rt) in enumerate(
                ((w1_hi, ee_hi), (w1_hi, ee_lo), (w1_lo, ee_hi))
            ):
                nc.tensor.matmul(
                    out=ps[:mw, :],
                    lhsT=lt[:, k * 128 : k * 128 + mw],
                    rhs=rt[:],
                    start=(term == 0),
                    stop=(term == 2),
                )
            nc.scalar.activation(
                out=hF[:mw, k, :],
                in_=ps[:mw, :],
                func=RELU,
                bias=b1_col[:mw, k : k + 1],
            )
            nc.vector.tensor_copy(out=hT_hi[:mw, k, :], in_=hF[:mw, k, :])
            nc.vector.tensor_tensor(
                out=hT_lo[:mw, k, :],
                in0=hF[:mw, k, :],
                in1=hT_hi[:mw, k, :],
                op=SUB,
            )

        # ---- MLP stage 2: ent = hT.T @ W2 + b2 ------------------------------
        # b2 enters the PSUM accumulation as a K=1 matmul of ones.T @ b2.
        # ent as bf16 hi/lo K-stacked [hi(0:32), lo(32:64)] per n-group, so
        # one K=64 bf16 matmul per scatter slice (stream cost unchanged,
        # FWL weight loads ~5x cheaper than f32r, precision ~2^-17 of ent).
        # The hi/lo split + partition-shift DMA pipelines per n-group while
        # the other group's accumulation still streams on the PE.
        entp = psB.tile([NE, H], f32)
        NGROUPS = ((0, 512), (512, H))
        # k-major interleave: both PSUM groups stream per k-chunk so they
        # finish together and the entHL splits start as early as possible.
        # 3-term bf16: hhi*w2hi + hlo*w2hi + hhi*w2lo; b2 via K=2 matmul.
        for n0, n1 in NGROUPS:
            nc.tensor.matmul(
                out=entp[:, n0:n1],
                lhsT=ones2[:],
                rhs=b2_hl[:, n0:n1],
                start=True,
                stop=False,
            )
        for k in range(KCH):
            whi, wlo = w2_q[k // KQ]
            koff = (k % KQ) * H
            for n0, n1 in NGROUPS:
                for term, (lt, rt) in enumerate(
                    ((hT_hi, whi), (hT_lo, whi), (hT_hi, wlo))
                ):
                    nc.tensor.matmul(
                        out=entp[:, n0:n1],
                        lhsT=lt[:, k, :],
                        rhs=rt[:, koff + n0 : koff + n1],
                        start=False,
                        stop=(k == KCH - 1 and term == 2),
                    )
        entHLs = {}
        for n0, n1 in NGROUPS:
            ehl = const.tile([2 * NE, n1 - n0], bf16, tag=f"entHL{n0}")
            elo = const.tile([NE, n1 - n0], bf16, tag=f"entlo{n0}")
            nc.scalar.copy(ehl[:NE, :], entp[:, n0:n1])  # cast f32 -> bf16 hi
            nc.vector.tensor_tensor(
                out=elo[:], in0=entp[:, n0:n1], in1=ehl[:NE, :], op=SUB
            )
            # partition shift 0:32 -> 32:64 via identity matmul (PE may write
            # base-32 psum; avoids an HWDGE DMA that starves behind gathers)
            pslo = psA.tile([2 * NE, n1 - n0], f32, tag="ps")
            nc.tensor.matmul(
                out=pslo[NE : 2 * NE, :],
                lhsT=id32_sb[:],
                rhs=elo[:],
                start=True,
                stop=True,
            )
            nc.scalar.copy(ehl[NE : 2 * NE, :], pslo[NE : 2 * NE, :])
            entHLs[n0] = ehl

        # ---- main loop: scatter-matmul, add, store -------------------------
        for g in range(NCH):
            gt = gts[g]
            sc = psC.tile([128, H], f32)
            for n0, n1 in ((0, 512), (512, H)):
                nc.tensor.matmul(
                    out=sc[:, n0:n1],
                    lhsT=maskT_sb[:, g * 128 : (g + 1) * 128],
                    rhs=entHLs[n0][:],
                    start=True,
                    stop=True,
                )
            ot = opool.tile([128, H], f32)
            nc.vector.tensor_add(ot[:], gt[:], sc[:])
            st_eng = nc.sync if g % 2 == 0 else nc.scalar
            st_eng.dma_start(out_ap[g * 128 : (g + 1) * 128, :], ot[:])

    nc.compile()
    return nc


def _get_program():
    global _PROGRAM
    if _PROGRAM is None:
        _PROGRAM = _build_program()
    return _PROGRAM


def _round_f32r(x):
    """Round f32 to the f32r (TF32-like) grid: 11 explicit mantissa bits, RNE."""
    x = np.ascontiguousarray(x, dtype=np.float32)
    xi = x.view(np.uint32).astype(np.uint64)
    shift = np.uint64(23 - 11)
    add = np.uint64(1) << np.uint64(23 - 11 - 1)
    xi2 = ((xi + add) >> shift) << shift
    return np.ascontiguousarray(xi2.astype(np.uint32).view(np.float32))


def _prep_shards(inputs):
    ids = np.ascontiguousarray(np.asarray(inputs["input_ids"]).astype(np.int32))
    ee = np.asarray(inputs["entity_embeddings"], dtype=np.float32)
    mask = np.asarray(inputs["entity_mask"], dtype=np.float32)
    we = np.ascontiguousarray(np.asarray(inputs["word_embedding"], dtype=np.float32))
    W1 = np.asarray(inputs["W1"], dtype=np.float32)
    b1 = np.asarray(inputs["b1"], dtype=np.float32)
    W2 = np.asarray(inputs["W2"], dtype=np.float32)
    b2 = np.asarray(inputs["b2"], dtype=np.float32)

    import ml_dtypes

    def split_hl(x):
        hi = x.astype(ml_dtypes.bfloat16)
        lo = (x - hi.astype(np.float32)).astype(ml_dtypes.bfloat16)
        return hi, lo

    w1_hi, w1_lo = split_hl(W1)  # [KG, MH] each
    w2_pad = np.concatenate([W2, np.zeros((KCH * 128 - MH, H), np.float32)], 0)
    w2p = w2_pad.reshape(KCH, 128, H).transpose(1, 0, 2).reshape(128, KCH * H)
    w2p_hi, w2p_lo = split_hl(w2p)
    w2p_hl = np.ascontiguousarray(np.concatenate([w2p_hi, w2p_lo], 1))
    b2_hi, b2_lo = split_hl(b2[None, :])
    ones_row = np.ones((1, NE), ml_dtypes.bfloat16)
    b2o = np.ascontiguousarray(
        np.concatenate(
            [
                np.concatenate([b2_hi, ones_row], 1),
                np.concatenate([b2_lo, ones_row], 1),
            ],
            0,
        )
    )  # [2, H+NE]
    b1pad = np.concatenate([b1, np.zeros(KCH * 128 - MH, np.float32)])
    b1colT = np.ascontiguousarray(b1pad.reshape(KCH, 128).T)  # [128, KCH]

    in_maps = []
    for i in range(NCORES):
        sl = slice(BPC * i, BPC * (i + 1))
        ids_i = ids[sl].reshape(-1)  # [TOK]
        idsT = np.ascontiguousarray(ids_i.reshape(NCH, 128).T)  # [128, NCH]
        ee_hi, ee_lo = split_hl(ee[sl].reshape(NE, KG).T)  # [KG, NE] each
        w1ee = np.ascontiguousarray(
            np.concatenate([w1_hi, w1_lo, ee_hi, ee_lo], 1)
        )  # [KG, 2*(MH+NE)]
        # block-diagonal [NE, TOK] mask, duplicated to [2*NE, TOK] so one
        # K=64 matmul covers both the hi and lo halves of entHL
        maskT = np.zeros((NE, TOK), np.float32)
        for b in range(BPC):
            maskT[b * E : (b + 1) * E, b * S : (b + 1) * S] = mask[BPC * i + b]
        maskT2 = np.ascontiguousarray(
            np.concatenate([maskT, maskT], 0).astype(ml_dtypes.bfloat16)
        )
        in_maps.append(
            {
                "idsT": idsT,
                "we": we,
                "w1ee": w1ee,
                "b1colT": b1colT,
                "w2p": w2p_hl,
                "b2o": b2o,
                "maskT2": maskT2,  # 0/1 values: exact in bf16
                "id32": np.eye(NE, dtype=ml_dtypes.bfloat16),
            }
        )
    return in_maps


def kernel(**inputs) -> np.ndarray:
    from concourse.bass_utils import run_bass_kernel_spmd

    trace = _maybe_enable_profiling()
    nc = _get_program()
    in_maps = _prep_shards(inputs)
    res = run_bass_kernel_spmd(
        nc, in_maps, core_ids=list(range(NCORES)), trace=trace
    )
    if trace and res.exec_time_ns is not None:
        print(f"HW exec time: {res.exec_time_ns} ns")
    out = np.concatenate(
        [res.results[i]["out"].reshape(BPC, S, H) for i in range(NCORES)], 0
    )
    return out


if __name__ == "__main__":
    rng = np.random.default_rng(0)
    inputs = {
        "input_ids": rng.integers(0, V, (B, S)).astype(np.int32),
        "entity_embeddings": rng.standard_normal((B, E, KG), dtype=np.float32),
        "entity_mask": (rng.random((B, E, S)) < 0.02).astype(np.float32),
        "word_embedding": rng.standard_normal((V, H), dtype=np.float32) * 0.02,
        "W1": rng.standard_normal((KG, MH), dtype=np.float32) * 0.02,
        "b1": np.zeros(MH, np.float32),
        "W2": rng.standard_normal((MH, H), dtype=np.float32) * 0.02,
        "b2": np.zeros(H, np.float32),
    }
    out = kernel(**inputs)
    ref = inputs["word_embedding"][inputs["input_ids"]] + np.einsum(
        "bes,beh->bsh",
        inputs["entity_mask"],
        np.maximum(
            inputs["entity_embeddings"] @ inputs["W1"] + inputs["b1"], 0.0
        )
        @ inputs["W2"]
        + inputs["b2"],
    )
    err = np.abs(out - ref).max() / max(np.abs(ref).max(), 1e-12)
    print("self-check rel err:", err)
